# revision 20
# baseline (speedup 1.0000x reference)
"""Triangle attention (starting node) Bass kernel for 8 trn2 NeuronCores.

Math (B=1, N=256, D=128, H=4, E=32):
  bias[h,j,k] = sum_d P[j,k,d] Wb[d,h]
  q[h,i,j,e]  = sum_d P[i,j,d] Wq[d,h*E+e]   (k,v analogous)
  S[i,h,j,k]  = (q . k) * E**-0.5 + bias[h,j,k]
  out[i,j,:]  = (softmax_k S @ v) merged over h, @ Wo

Sharding: rows i are split across 8 cores (32 rows each). The bias couples all
rows; each core computes the bias columns for its 32 local j-rows, an on-device
AllGather assembles the full [h, j, k] bias, and attention then runs per row
shard — all in ONE program / one PJRT dispatch (dispatch overhead through the
axon tunnel dominates end-to-end time, so the two-dispatch host-gather variant
pays twice).

On-chip layout is "T-form": scores are built transposed, ST[k, j] per head, so
softmax normalization sums over the partition axis (done on the PE with a ones
matmul, replicated x32 for free) and the AV matmul consumes ST directly with
no transpose of the attention matrix. The host supplies pairwise_repr already
transposed to [d, i*N+token] so every on-chip matmul operand has its
contraction axis on partitions.
"""

from contextlib import ExitStack

import numpy as np

N = 256
D = 128
H = 4
E = 32
NCORES = 8
RPC = N // NCORES  # rows per core
SCALE = float(E) ** -0.5

_cache = {}


def _dt():
    import concourse.mybir as mybir

    return mybir.dt.float32


_legal_ctr = [0]


def _legalize_waits(nc):
    """Walrus caps semaphore wait-commands per lowered instruction (LDWEIGHTS
    holds only one). Hoist excess waits of every non-Drain instruction into
    fresh single-wait NoOps on the same engine, inserted right before it —
    same wait point, so timing/deadlock semantics are unchanged."""
    import bass_rust

    for fn in nc.m.functions:
        for blk in fn.blocks:
            ins = blk.instructions
            i = 0
            while i < len(ins):
                inst = ins[i]
                si = inst.sync_info
                if si is None or inst.engine is None:
                    i += 1
                    continue
                waits = si.on_wait
                if len(waits) <= 1:
                    i += 1
                    continue
                for w in waits[:-1]:
                    _legal_ctr[0] += 1
                    n = bass_rust.InstNoOp(name=f"I-lgl-{_legal_ctr[0]}")
                    n.engine = inst.engine
                    n.sync_info = bass_rust.SyncInfo(on_wait=[w], on_update=[])
                    ins.insert(i, n)
                    i += 1
                si.on_wait = [waits[-1]]
                inst.sync_info = si
                i += 1


def _build_fused(reps=1):
    """One program per core: local bias columns -> AllGather -> attention.

    reps>1 unrolls the whole body back-to-back (same inputs/outputs) — used
    only by timing probes to difference out dispatch overhead."""
    import concourse.bass as bass
    import concourse.mybir as mybir
    import concourse.tile as tile
    from concourse.masks import make_identity

    f32 = _dt()
    AF = mybir.ActivationFunctionType
    nc = bass.Bass("TRN2", target_bir_lowering=False, debug=False,
                   enable_asserts=False, num_devices=NCORES)
    xT = nc.dram_tensor("xT", [D, RPC * N], f32, kind="ExternalInput").ap()
    wb = nc.dram_tensor("wb", [D, H], f32, kind="ExternalInput").ap()
    wq = nc.dram_tensor("wq", [D, D], f32, kind="ExternalInput").ap()
    wk = nc.dram_tensor("wk", [D, D], f32, kind="ExternalInput").ap()
    wv = nc.dram_tensor("wv", [D, D], f32, kind="ExternalInput").ap()
    wo = nc.dram_tensor("wo", [D, D], f32, kind="ExternalInput").ap()
    outT = nc.dram_tensor("outT", [RPC, D, N], f32, kind="ExternalOutput").ap()

    RB = 4  # rows per projection batch
    with ExitStack() as ctx:
        tc = ctx.enter_context(tile.TileContext(nc))
        singles = ctx.enter_context(tc.tile_pool(name="singles", bufs=1))
        dram = ctx.enter_context(tc.tile_pool(name="dram", bufs=1, space="DRAM"))
        qk_pool = ctx.enter_context(tc.tile_pool(name="qk", bufs=2))
        v_pool = ctx.enter_context(tc.tile_pool(name="v", bufs=3))
        es_pool = ctx.enter_context(tc.tile_pool(name="es", bufs=4))
        e0_pool = ctx.enter_context(tc.tile_pool(name="e0", bufs=3))
        sm_pool = ctx.enter_context(tc.tile_pool(name="sm", bufs=3))
        out_pool = ctx.enter_context(tc.tile_pool(name="outp", bufs=3))
        s_psum = ctx.enter_context(tc.tile_pool(name="spsum", bufs=2, space="PSUM"))
        o_psum = ctx.enter_context(tc.tile_pool(name="opsum", bufs=2, space="PSUM"))
        m_psum = ctx.enter_context(tc.tile_pool(name="mpsum", bufs=2, space="PSUM"))

        wb_sb = singles.tile([D, H], f32)
        wq_sb = singles.tile([D, D], f32)
        wk_sb = singles.tile([D, D], f32)
        wv_sb = singles.tile([D, D], f32)
        wo_sb = singles.tile([D, D], f32)
        ones = singles.tile([128, E], f32)
        bias_sb = singles.tile([128, 2 * H * N], f32)  # [k, kh*1024 + h*256 + j]
        ebias_sb = singles.tile([128, 2 * H * N], f32)  # exp(bias)
        xt_sb = singles.tile([D, RPC * N], f32)

        nc.sync.dma_start(out=wb_sb, in_=wb)
        nc.sync.dma_start(out=wq_sb, in_=wq)
        nc.sync.dma_start(out=wk_sb, in_=wk)
        nc.sync.dma_start(out=wv_sb, in_=wv)
        nc.sync.dma_start(out=wo_sb, in_=wo)
        nc.vector.memset(ones, 1.0)
        for c in range(8):
            sl = slice(c * RPC * N // 8, (c + 1) * RPC * N // 8)
            nc.sync.dma_start(out=xt_sb[:, sl], in_=xT[:, sl])

        # --- phase A: local bias columns bshard[kh, k, h*RPC + jl]
        bshard = dram.tile([2, 128, H * RPC], f32)
        bgather = dram.tile([NCORES, 2, 128, H * RPC], f32)
        st = singles.tile([128, 2 * H * RPC], f32)

        def body():
         for kh in range(2):
            pb = m_psum.tile([128, RPC * H], f32, tag="m")  # [k, jl*H + h]
            for jl in range(RPC):
                nc.tensor.matmul(
                    pb[:, jl * H:(jl + 1) * H],
                    xt_sb[:, jl * N + kh * 128: jl * N + kh * 128 + 128],
                    wb_sb,
                    start=True, stop=True,
                )
            # st[:, kh*128 + h*RPC + jl] = pb[:, jl*H + h]
            nc.vector.tensor_copy(
                st[:, kh * H * RPC:(kh + 1) * H * RPC].rearrange(
                    "p (h j) -> p h j", h=H),
                pb.rearrange("p (j h) -> p h j", h=H),
            )
         for kh in range(2):
            nc.sync.dma_start(out=bshard[kh],
                              in_=st[:, kh * H * RPC:(kh + 1) * H * RPC])

         # --- AllGather bias shards across the 8 cores
         nc.gpsimd.collective_compute(
            "AllGather",
            mybir.AluOpType.bypass,
            replica_groups=[list(range(NCORES))],
            ins=[bshard[:].opt()],
            outs=[bgather[:].opt()],
         )

         # --- load full bias: bias_sb[k, kh*H*N + h*N + c*RPC + jl]
         for kh in range(2):
            src = bgather[:, kh].rearrange("c k (h j) -> k h c j", h=H)
            dst = bias_sb[:, kh * H * N:(kh + 1) * H * N].rearrange(
                "p (h c j) -> p h c j", h=H, c=NCORES)
            nc.sync.dma_start(out=dst, in_=src)
         # exp(bias) once: scores use exp(qk+bias) = exp(qk) * exp(bias), so
         # the per-row bias broadcast costs a vector multiply instead of a
         # PE ident-matmul into PSUM.
         for kh in range(2):
            nc.scalar.activation(
                ebias_sb[:, kh * H * N:(kh + 1) * H * N],
                bias_sb[:, kh * H * N:(kh + 1) * H * N], AF.Exp)

         # --- phase B: attention over the 32 local rows
         for rb in range(RPC // RB):
            # projections for RB rows, per-head partition-0 aligned:
            # qT/kT[e, h*RB*N + rb-local row * N + token]. Keeping every head
            # at partitions 0-31 lets the score matmuls open their own PSUM
            # accumulation groups at tile_position (0,0) — the nonzero-row-
            # offset start form aborts the NEFF at runtime.
            qT = qk_pool.tile([32, H * RB * N], f32, tag="qT")
            kT = qk_pool.tile([32, H * RB * N], f32, tag="kT")
            for m, (wsb, dst, scl) in enumerate(
                    [(wq_sb, qT, SCALE), (wk_sb, kT, 1.0)]):
                for c in range(RB * N // 512):
                    for h in range(H):
                        pp = m_psum.tile([32, 512], f32, tag="m")
                        nc.tensor.matmul(
                            pp,
                            wsb[:, 32 * h:32 * h + 32],
                            xt_sb[:, rb * RB * N + c * 512: rb * RB * N + (c + 1) * 512],
                            start=True, stop=True)
                        dsl = dst[:, h * RB * N + c * 512: h * RB * N + (c + 1) * 512]
                        if scl == 1.0:
                            nc.vector.tensor_copy(dsl, pp)
                        else:
                            nc.vector.tensor_scalar_mul(dsl, pp, scl)

            for rl in range(RB):
                r = rb * RB + rl
                roff = rb * RB * N + rl * N
                # --- v for this row: v_sb[ktok, half*128 + he]
                v_sb = v_pool.tile([128, N], f32, tag="v")
                pv = m_psum.tile([128, 512], f32, tag="m")
                for half in range(2):
                    nc.tensor.matmul(
                        pv[:, half * 128:(half + 1) * 128],
                        xt_sb[:, roff + half * 128: roff + half * 128 + 128],
                        wv_sb,
                        start=True, stop=True)
                nc.vector.tensor_copy(v_sb, pv[:, 0:N])

                # --- scores + exp, per k-half chunk [128, H*N]
                est = []
                for kh in range(2):
                    sp = s_psum.tile([128, H * N], f32, tag="s")
                    for h in range(H):
                        ho = h * RB * N + rl * N
                        nc.tensor.matmul(
                            sp[:, h * N:(h + 1) * N],
                            kT[:, ho + kh * 128: ho + kh * 128 + 128],
                            qT[:, ho: ho + N],
                            start=True, stop=True)
                    e0 = e0_pool.tile([128, H * N], f32, tag="e0")
                    nc.scalar.activation(e0, sp, AF.Exp)
                    es = es_pool.tile([128, H * N], f32, tag="es")
                    nc.vector.tensor_mul(
                        es, e0, ebias_sb[:, kh * H * N:(kh + 1) * H * N])
                    est.append(es)

                # --- rowsums (replicated x32 via ones[128,E]) and AV
                po = o_psum.tile([128, 512], f32, tag="o")
                for h in range(H):
                    for kh in range(2):
                        nc.tensor.matmul(
                            po[32 * h:32 * h + 32, 256:512],
                            ones,
                            est[kh][:, h * N:(h + 1) * N],
                            start=(kh == 0), stop=(kh == 1),
                            tile_position=(0, 32 * h))
                for h in range(H):
                    for kh in range(2):
                        nc.tensor.matmul(
                            po[32 * h:32 * h + 32, 0:256],
                            v_sb[:, kh * 128 + 32 * h: kh * 128 + 32 * h + 32],
                            est[kh][:, h * N:(h + 1) * N],
                            start=(kh == 0), stop=(kh == 1),
                            tile_position=(0, 32 * h))

                rs_rec = sm_pool.tile([128, N], f32, tag="rs")
                nc.vector.reciprocal(rs_rec, po[:, 256:512])
                oT_sb = sm_pool.tile([128, N], f32, tag="oT")
                nc.vector.tensor_mul(oT_sb, po[:, 0:256], rs_rec)

                # --- output projection: outT[d, j] = sum_he Wo[he,d] oT[he,j]
                pf = m_psum.tile([128, 512], f32, tag="m")
                nc.tensor.matmul(pf[:, 0:N], wo_sb, oT_sb, start=True, stop=True)
                o_sb = out_pool.tile([128, N], f32, tag="osb")
                nc.vector.tensor_copy(o_sb, pf[:, 0:N])
                nc.sync.dma_start(out=outT[r], in_=o_sb)

        for _ in range(reps):
            body()
    return nc


def _get_program():
    if "nc" not in _cache:
        _cache["nc"] = _build_fused()
        _legalize_waits(_cache["nc"])
    return _cache["nc"]


def kernel(pairwise_repr, mask, Wb, Wq, Wk, Wv, Wo):
    from concourse.bass_utils import run_bass_kernel_spmd

    nc = _get_program()

    x = np.ascontiguousarray(np.asarray(pairwise_repr, dtype=np.float32)[0])
    # xT[d, i*N + t] = x[i, t, d]
    xT = np.ascontiguousarray(x.reshape(N * N, D).T)
    shards = [np.ascontiguousarray(xT[:, c * RPC * N:(c + 1) * RPC * N])
              for c in range(NCORES)]
    wb = np.ascontiguousarray(np.asarray(Wb, np.float32))
    wq = np.ascontiguousarray(np.asarray(Wq, np.float32))
    wk = np.ascontiguousarray(np.asarray(Wk, np.float32))
    wv = np.ascontiguousarray(np.asarray(Wv, np.float32))
    wo = np.ascontiguousarray(np.asarray(Wo, np.float32))

    core_ids = list(range(NCORES))
    in_maps = [{"xT": shards[c], "wb": wb, "wq": wq, "wk": wk,
                "wv": wv, "wo": wo} for c in range(NCORES)]
    kernel._last_in = in_maps
    res = run_bass_kernel_spmd(nc, in_maps, core_ids=core_ids, trace=False)

    kernel._last = res
    # outT [RPC, D, N] per core -> out[0, 32c+r, j, d] = outT_c[r, d, j]
    o = np.stack([res.results[c]["outT"] for c in range(NCORES)])
    out = o.transpose(0, 1, 3, 2).reshape(1, N, N, D)
    return np.ascontiguousarray(out.astype(np.float32))


# revision 25
# speedup vs baseline: 1.6581x; 1.6581x over previous
"""Triangle attention (starting node) Bass kernel for 8 trn2 NeuronCores.

Math (B=1, N=256, D=128, H=4, E=32):
  bias[h,j,k] = sum_d P[j,k,d] Wb[d,h]
  q[h,i,j,e]  = sum_d P[i,j,d] Wq[d,h*E+e]   (k,v analogous)
  S[i,h,j,k]  = (q . k) * E**-0.5 + bias[h,j,k]
  out[i,j,:]  = (softmax_k S @ v) merged over h, @ Wo

Sharding: rows i are split across 8 cores (32 rows each). The bias couples all
rows; each core computes the bias columns for its 32 local j-rows, an on-device
AllGather assembles the full [h, j, k] bias, and attention then runs per row
shard — all in ONE program / one PJRT dispatch (dispatch overhead through the
axon tunnel dominates end-to-end time, so the two-dispatch host-gather variant
pays twice).

On-chip layout is "T-form": scores are built transposed, ST[k, j] per head, so
softmax normalization sums over the partition axis (done on the PE with a ones
matmul, replicated x32 for free) and the AV matmul consumes ST directly with
no transpose of the attention matrix. The host supplies pairwise_repr already
transposed to [d, i*N+token] so every on-chip matmul operand has its
contraction axis on partitions.
"""

from contextlib import ExitStack

import numpy as np

N = 256
D = 128
H = 4
E = 32
NCORES = 8
RPC = N // NCORES  # rows per core
SCALE = float(E) ** -0.5

_cache = {}


def _dt():
    import concourse.mybir as mybir

    return mybir.dt.float32


_legal_ctr = [0]


def _legalize_waits(nc):
    """Walrus caps semaphore wait-commands per lowered instruction (LDWEIGHTS
    holds only one). Hoist excess waits of every non-Drain instruction into
    fresh single-wait NoOps on the same engine, inserted right before it —
    same wait point, so timing/deadlock semantics are unchanged."""
    import bass_rust

    for fn in nc.m.functions:
        for blk in fn.blocks:
            ins = blk.instructions
            i = 0
            while i < len(ins):
                inst = ins[i]
                si = inst.sync_info
                if si is None or inst.engine is None:
                    i += 1
                    continue
                waits = si.on_wait
                if len(waits) <= 1:
                    i += 1
                    continue
                for w in waits[:-1]:
                    _legal_ctr[0] += 1
                    n = bass_rust.InstNoOp(name=f"I-lgl-{_legal_ctr[0]}")
                    n.engine = inst.engine
                    n.sync_info = bass_rust.SyncInfo(on_wait=[w], on_update=[])
                    ins.insert(i, n)
                    i += 1
                si.on_wait = [waits[-1]]
                inst.sync_info = si
                i += 1


def _build_fused(reps=1):
    """One program per core: local bias columns -> AllGather -> attention.

    reps>1 unrolls the whole body back-to-back (same inputs/outputs) — used
    only by timing probes to difference out dispatch overhead."""
    import concourse.bass as bass
    import concourse.mybir as mybir
    import concourse.tile as tile
    from concourse.masks import make_identity

    f32 = _dt()
    AF = mybir.ActivationFunctionType
    nc = bass.Bass("TRN2", target_bir_lowering=False, debug=False,
                   enable_asserts=False, num_devices=NCORES)
    xT = nc.dram_tensor("xT", [D, RPC * N], f32, kind="ExternalInput").ap()
    wb = nc.dram_tensor("wb", [D, H], f32, kind="ExternalInput").ap()
    wq = nc.dram_tensor("wq", [D, D], f32, kind="ExternalInput").ap()
    wk = nc.dram_tensor("wk", [D, D], f32, kind="ExternalInput").ap()
    wv = nc.dram_tensor("wv", [D, D], f32, kind="ExternalInput").ap()
    wo = nc.dram_tensor("wo", [D, D], f32, kind="ExternalInput").ap()
    outT = nc.dram_tensor("outT", [RPC, D, N], f32, kind="ExternalOutput").ap()

    RB = 4  # rows per projection batch
    with ExitStack() as ctx:
        tc = ctx.enter_context(tile.TileContext(nc))
        singles = ctx.enter_context(tc.tile_pool(name="singles", bufs=1))
        dram = ctx.enter_context(tc.tile_pool(name="dram", bufs=1, space="DRAM"))
        qk_pool = ctx.enter_context(tc.tile_pool(name="qk", bufs=2))
        v_pool = ctx.enter_context(tc.tile_pool(name="v", bufs=3))
        es_pool = ctx.enter_context(tc.tile_pool(name="es", bufs=4))
        e0_pool = ctx.enter_context(tc.tile_pool(name="e0", bufs=3))
        sm_pool = ctx.enter_context(tc.tile_pool(name="sm", bufs=3))
        out_pool = ctx.enter_context(tc.tile_pool(name="outp", bufs=3))
        s_psum = ctx.enter_context(tc.tile_pool(name="spsum", bufs=2, space="PSUM"))
        o_psum = ctx.enter_context(tc.tile_pool(name="opsum", bufs=2, space="PSUM"))
        m_psum = ctx.enter_context(tc.tile_pool(name="mpsum", bufs=2, space="PSUM"))

        wb_sb = singles.tile([D, H], f32)
        wq_sb = singles.tile([D, D], f32)
        wk_sb = singles.tile([D, D], f32)
        wv_sb = singles.tile([D, D], f32)
        wo_sb = singles.tile([D, D], f32)
        ones = singles.tile([128, E], f32)
        bias_sb = singles.tile([128, 2 * H * N], f32)  # [k, kh*1024 + h*256 + j]
        ebias_sb = singles.tile([128, 2 * H * N], f32)  # exp(bias)
        xt_sb = singles.tile([D, RPC * N], f32)

        nc.sync.dma_start(out=wb_sb, in_=wb)
        nc.sync.dma_start(out=wq_sb, in_=wq)
        nc.sync.dma_start(out=wk_sb, in_=wk)
        nc.sync.dma_start(out=wv_sb, in_=wv)
        nc.sync.dma_start(out=wo_sb, in_=wo)
        nc.vector.memset(ones, 1.0)
        for c in range(8):
            sl = slice(c * RPC * N // 8, (c + 1) * RPC * N // 8)
            nc.sync.dma_start(out=xt_sb[:, sl], in_=xT[:, sl])

        # --- phase A: local bias columns bshard[kh, k, h*RPC + jl]
        bshard = dram.tile([2, 128, H * RPC], f32)
        bgather = dram.tile([NCORES, 2, 128, H * RPC], f32)
        st = singles.tile([128, 2 * H * RPC], f32)

        def body():
         for kh in range(2):
            pb = m_psum.tile([128, RPC * H], f32, tag="m")  # [k, jl*H + h]
            for jl in range(RPC):
                nc.tensor.matmul(
                    pb[:, jl * H:(jl + 1) * H],
                    xt_sb[:, jl * N + kh * 128: jl * N + kh * 128 + 128],
                    wb_sb,
                    start=True, stop=True,
                )
            # st[:, kh*128 + h*RPC + jl] = pb[:, jl*H + h]
            nc.vector.tensor_copy(
                st[:, kh * H * RPC:(kh + 1) * H * RPC].rearrange(
                    "p (h j) -> p h j", h=H),
                pb.rearrange("p (j h) -> p h j", h=H),
            )
         for kh in range(2):
            nc.sync.dma_start(out=bshard[kh],
                              in_=st[:, kh * H * RPC:(kh + 1) * H * RPC])

         # --- AllGather bias shards across the 8 cores
         nc.gpsimd.collective_compute(
            "AllGather",
            mybir.AluOpType.bypass,
            replica_groups=[list(range(NCORES))],
            ins=[bshard[:].opt()],
            outs=[bgather[:].opt()],
         )

         # --- load full bias: bias_sb[k, kh*H*N + h*N + c*RPC + jl]
         for kh in range(2):
            src = bgather[:, kh].rearrange("c k (h j) -> k h c j", h=H)
            dst = bias_sb[:, kh * H * N:(kh + 1) * H * N].rearrange(
                "p (h c j) -> p h c j", h=H, c=NCORES)
            nc.sync.dma_start(out=dst, in_=src)
         # exp(bias) once: scores use exp(qk+bias) = exp(qk) * exp(bias), so
         # the per-row bias broadcast costs a vector multiply instead of a
         # PE ident-matmul into PSUM.
         for kh in range(2):
            nc.scalar.activation(
                ebias_sb[:, kh * H * N:(kh + 1) * H * N],
                bias_sb[:, kh * H * N:(kh + 1) * H * N], AF.Exp)

         # --- phase B: attention over the 32 local rows
         for rb in range(RPC // RB):
            # projections for RB rows, per-head partition-0 aligned:
            # qT/kT[e, h*RB*N + rb-local row * N + token]. Keeping every head
            # at partitions 0-31 lets the score matmuls open their own PSUM
            # accumulation groups at tile_position (0,0) — the nonzero-row-
            # offset start form aborts the NEFF at runtime.
            qT = qk_pool.tile([32, H * RB * N], f32, tag="qT")
            kT = qk_pool.tile([32, H * RB * N], f32, tag="kT")
            for m, (wsb, dst, scl) in enumerate(
                    [(wq_sb, qT, SCALE), (wk_sb, kT, 1.0)]):
                for c in range(RB * N // 512):
                    for h in range(H):
                        pp = m_psum.tile([32, 512], f32, tag="m")
                        nc.tensor.matmul(
                            pp,
                            wsb[:, 32 * h:32 * h + 32],
                            xt_sb[:, rb * RB * N + c * 512: rb * RB * N + (c + 1) * 512],
                            start=True, stop=True)
                        dsl = dst[:, h * RB * N + c * 512: h * RB * N + (c + 1) * 512]
                        if scl == 1.0:
                            nc.vector.tensor_copy(dsl, pp)
                        else:
                            nc.vector.tensor_scalar_mul(dsl, pp, scl)

            for rl in range(RB):
                r = rb * RB + rl
                roff = rb * RB * N + rl * N
                # --- v for this row: v_sb[ktok, half*128 + he]
                v_sb = v_pool.tile([128, N], f32, tag="v")
                pv = m_psum.tile([128, 512], f32, tag="m")
                for half in range(2):
                    nc.tensor.matmul(
                        pv[:, half * 128:(half + 1) * 128],
                        xt_sb[:, roff + half * 128: roff + half * 128 + 128],
                        wv_sb,
                        start=True, stop=True)
                nc.vector.tensor_copy(v_sb, pv[:, 0:N])

                # --- scores + exp, per k-half chunk [128, H*N]
                est = []
                for kh in range(2):
                    sp = s_psum.tile([128, H * N], f32, tag="s")
                    for h in range(H):
                        ho = h * RB * N + rl * N
                        nc.tensor.matmul(
                            sp[:, h * N:(h + 1) * N],
                            kT[:, ho + kh * 128: ho + kh * 128 + 128],
                            qT[:, ho: ho + N],
                            start=True, stop=True)
                    e0 = e0_pool.tile([128, H * N], f32, tag="e0")
                    nc.scalar.activation(e0, sp, AF.Exp)
                    es = es_pool.tile([128, H * N], f32, tag="es")
                    nc.vector.tensor_mul(
                        es, e0, ebias_sb[:, kh * H * N:(kh + 1) * H * N])
                    est.append(es)

                # --- rowsums (replicated x32 via ones[128,E]) and AV
                po = o_psum.tile([128, 512], f32, tag="o")
                for h in range(H):
                    for kh in range(2):
                        nc.tensor.matmul(
                            po[32 * h:32 * h + 32, 256:512],
                            ones,
                            est[kh][:, h * N:(h + 1) * N],
                            start=(kh == 0), stop=(kh == 1),
                            tile_position=(0, 32 * h))
                for h in range(H):
                    for kh in range(2):
                        nc.tensor.matmul(
                            po[32 * h:32 * h + 32, 0:256],
                            v_sb[:, kh * 128 + 32 * h: kh * 128 + 32 * h + 32],
                            est[kh][:, h * N:(h + 1) * N],
                            start=(kh == 0), stop=(kh == 1),
                            tile_position=(0, 32 * h))

                rs_rec = sm_pool.tile([128, N], f32, tag="rs")
                nc.vector.reciprocal(rs_rec, po[:, 256:512])
                oT_sb = sm_pool.tile([128, N], f32, tag="oT")
                nc.vector.tensor_mul(oT_sb, po[:, 0:256], rs_rec)

                # --- output projection: outT[d, j] = sum_he Wo[he,d] oT[he,j]
                pf = m_psum.tile([128, 512], f32, tag="m")
                nc.tensor.matmul(pf[:, 0:N], wo_sb, oT_sb, start=True, stop=True)
                o_sb = out_pool.tile([128, N], f32, tag="osb")
                nc.vector.tensor_copy(o_sb, pf[:, 0:N])
                nc.sync.dma_start(out=outT[r], in_=o_sb)

        for _ in range(reps):
            body()
    return nc


def _get_program():
    if "nc" not in _cache:
        _cache["nc"] = _build_fused()
        _legalize_waits(_cache["nc"])
    return _cache["nc"]


def kernel(pairwise_repr, mask, Wb, Wq, Wk, Wv, Wo):
    from concourse.bass_utils import run_bass_kernel_spmd

    nc = _get_program()

    x = np.ascontiguousarray(np.asarray(pairwise_repr, dtype=np.float32)[0])
    # xT[d, i*N + t] = x[i, t, d]
    xT = np.ascontiguousarray(x.reshape(N * N, D).T)
    shards = [np.ascontiguousarray(xT[:, c * RPC * N:(c + 1) * RPC * N])
              for c in range(NCORES)]
    wb = np.ascontiguousarray(np.asarray(Wb, np.float32))
    wq = np.ascontiguousarray(np.asarray(Wq, np.float32))
    wk = np.ascontiguousarray(np.asarray(Wk, np.float32))
    wv = np.ascontiguousarray(np.asarray(Wv, np.float32))
    wo = np.ascontiguousarray(np.asarray(Wo, np.float32))

    core_ids = list(range(NCORES))
    in_maps = [{"xT": shards[c], "wb": wb, "wq": wq, "wk": wk,
                "wv": wv, "wo": wo} for c in range(NCORES)]
    kernel._last_in = in_maps
    res = run_bass_kernel_spmd(nc, in_maps, core_ids=core_ids, trace=False)

    kernel._last = res
    # outT [RPC, D, N] per core -> out[0, 32c+r, j, d] = outT_c[r, d, j]
    o = np.stack([res.results[c]["outT"] for c in range(NCORES)])
    out = o.transpose(0, 1, 3, 2).reshape(1, N, N, D)
    return np.ascontiguousarray(out.astype(np.float32))
